# revision 9
# baseline (speedup 1.0000x reference)
"""FlowNetC correlation (nn_Correlation_27797028340332) on 8 TRN2 NeuronCores.

out[b, dy*21+dx, y, x] = mean_c in1[b,c,y,x] * in2p[b,c,y+2*dy, x+2*dx]
with in2p = zero-pad(in2, 20) and (dy, dx) over a 21x21 stride-2 grid.

Strategy (per core; data-parallel over batch B=8):
  - Inputs are cast to fp16 on the host and DMA'd straight into resident
    SBUF tiles (no on-chip cast), halving input HBM traffic vs fp32.
  - The per-pixel C=256 dot products are banded Gram matmuls on the
    TensorEngine, 4x column-tiled: the 128 stationary columns are split
    into 4 tiles of 32 (parity p x x-block b), each with its OWN moving
    stream covering only the 52 in2p columns its 21-wide output band
    needs (vs 84 for a 64-wide tile).  The 4 tiles run concurrently in
    the PE array (tile_position=(0,32t)), so a (y,dy) pair costs ~52
    moving columns per contraction chunk instead of 84, and the dumped
    Gram rectangle shrinks from 84 to 52 columns per displacement.
  - PSUM: per (y, dy-batch<=7) one 2KB bank holds [128, 52*bsz] fp32.
    A 1-column dummy matmul with start=True clears the bank's
    has_written bits (bank-wide clear), then all 8 real matmuls (4 tiles
    x 2 contraction chunks) run with start=False: first write to each
    element overwrites, second accumulates.  This permits concurrent
    tiles to share a bank without clobbering each other's bits.
  - The needed output band G[x, x+2k] (k=0..20) is a per-partition
    diagonal no engine can extract at line rate, so the 52-column Grams
    are cast to fp16 (DVE/ACT alternating) and dumped to DRAM; the shear
    is a numpy strided view on the host inside kernel().
"""

import numpy as np

B, C, H, W = 8, 256, 96, 128
PAD = 20
D = 21            # displacements per axis
CH = 2            # contraction chunks of 128
WPAD = W + 2 * PAD    # 168
XB = 32           # stationary columns per PE tile
JW = 52           # moving columns per tile (32 outputs + 20 band overhang)
NT = 4            # PE column tiles (2 parities x 2 x-blocks)
BSZ = 7           # dy's per PSUM bank (7*52 = 364 <= 512 fp32 bank)
NBANK = 3         # PSUM banks per y (ceil(21/7))
ROWBLK = 8
N_CORES = 8


def _valid_dys(y):
    """dy' indices with in-range source row y2 = y + 2*dy' - 20."""
    return [d for d in range(D) if 0 <= y + 2 * d - PAD < H]


def _batches(n):
    """Chunks of BSZ with remainder tail: n=21 -> (7,7,7); n=11 -> (7,4)."""
    out = [BSZ] * (n // BSZ)
    if n % BSZ:
        out.append(n % BSZ)
    return out


def _dump_layout():
    """Per-y (n_dy, element offset) layout of the dump tensor's free dim."""
    offs, off = [], 0
    for y in range(H):
        n = len(_valid_dys(y))
        offs.append((n, off))
        off += n * JW
    return offs, off


_NC_CACHE = {}


def _build(reps=1):
    import contextlib

    import concourse.bacc as bacc
    import concourse.tile as tile
    import concourse.tile_rust as tile_rust
    from concourse import mybir

    offs, total = _dump_layout()

    nc = bacc.Bacc("TRN2", target_bir_lowering=False, debug=False)
    in1_d = nc.dram_tensor("in1", [C, H, W], mybir.dt.float16,
                           kind="ExternalInput").ap()
    in2_d = nc.dram_tensor("in2", [C, H, W], mybir.dt.float16,
                           kind="ExternalInput").ap()
    dump_d = nc.dram_tensor("dump", [128, total], mybir.dt.float16,
                            kind="ExternalOutput").ap()

    YGRP = 4                      # y rows per dump DMA
    MAXW = NBANK * BSZ * JW       # 1092 elems: worst-case per-y dump width

    with tile.TileContext(nc) as tc:
        with tc.tile_pool(name="resident", bufs=1) as res_pool, \
             tc.tile_pool(name="out", bufs=4) as out_pool, \
             tc.tile_pool(name="psum", bufs=8, space="PSUM") as psum_pool, \
             (tc.For_i(0, reps, 1) if reps > 1 else contextlib.nullcontext()):

            # Fully-resident fp16 feature maps; in2 zero-padded along x.
            in1s = res_pool.tile([128, CH, H, W], mybir.dt.float16)
            in2p = res_pool.tile([128, CH, H, WPAD], mybir.dt.float16)
            # Pad memsets split so y=0 only waits on the small rows-0-15
            # piece (~1.4us on DVE), not a whole-tensor gpsimd memset.
            PRE = 2 * ROWBLK
            nc.vector.memset(in2p[:, :, 0:PRE, 0:PAD], 0.0)
            nc.vector.memset(in2p[:, :, 0:PRE, W + PAD:WPAD], 0.0)
            nc.gpsimd.memset(in2p[:, :, PRE:H, 0:PAD], 0.0)
            nc.gpsimd.memset(in2p[:, :, PRE:H, W + PAD:WPAD], 0.0)

            def load1_block(yb, eng=None):
                y0 = yb * ROWBLK
                (eng or nc.gpsimd).dma_start(
                    in1s[:, :, y0:y0 + ROWBLK, :],
                    in1_d[:, y0:y0 + ROWBLK, :].rearrange(
                        "(k p) y x -> p k y x", p=128))

            def load2_block(yb, eng=None):
                y0 = yb * ROWBLK
                for k in range(CH):
                    (eng or nc.gpsimd).dma_start(
                        in2p[:, k, y0:y0 + ROWBLK, PAD:PAD + W],
                        in2_d[128 * k:128 * (k + 1), y0:y0 + ROWBLK, :]
                        .rearrange("p y x -> p y x"))

            def load_block(yb):
                """DMA rows [yb*8, yb*8+8) of both inputs into SBUF."""
                load1_block(yb)
                load2_block(yb)

            # Prologue on the idle HWDGE rings (SP + ACT), y=0's deps
            # first (in1 rows 0-7, in2 rows 0-23), then the rest of the
            # 4-block lookahead window; steady-state loads use SWDGE.
            # (Splitting the first in1 load to row-0-only starts y=0
            # ~1.1us earlier but starves y=1-4 — net worse; measured.)
            load1_block(0, nc.sync)
            load2_block(0, nc.scalar)
            load2_block(1, nc.sync)
            load2_block(2, nc.scalar)
            load2_block(3, nc.sync)
            for yb in range(1, 4):
                load1_block(yb)

            def copy_dve(out, in_):
                nc.vector.tensor_copy(out, in_)

            def copy_act(out, in_):
                nc.scalar.copy(out, in_)

            # Dump groups of 4 y's, except the last 8 y's in pairs so the
            # final copies+DMA tail after the last matmul stays short.
            gstart = set(range(0, H - 8, YGRP)) | set(range(H - 8, H, 2))
            gend = {y - 1 for y in gstart if y > 0} | {H - 1}

            stage = None
            goff = 0
            gbase = 0
            for y in range(H):
                # Stay 3-4 blocks ahead of the in2 read frontier (y+20).
                if y % ROWBLK == 0:
                    yb = y // ROWBLK + 4
                    if yb < H // ROWBLK:
                        load_block(yb)

                if y in gstart:
                    stage = out_pool.tile([128, YGRP * MAXW],
                                          mybir.dt.float16, tag="dumpstage")
                    goff = 0
                    gbase = offs[y][1]

                dys = _valid_dys(y)
                n_dy = len(dys)
                bs = _batches(n_dy)

                di = 0
                for ib, bsz in enumerate(bs):
                    # One PSUM bank per dy-batch, 6 in flight: matmuls for
                    # later batches never wait on this batch's evacuation,
                    # so the PE stays busy (and in its fast p-state) while
                    # DVE/ACT drain earlier banks.
                    ps = psum_pool.tile([128, 512], mybir.dt.float32,
                                        tag="ps")
                    dy0 = dys[di]
                    y2f = y + 2 * dy0 - PAD
                    # Dummy 1-col matmuls, one per column tile: start=True
                    # clears the bank's has_written bits (their union spans
                    # all 128 partitions); they write only col 511 (never
                    # read).  Real matmuls then use start=False so the 4
                    # concurrent column tiles can share the bank.  Col-tiled
                    # dummies chain behind their own group's stream instead
                    # of barriering the whole array like a 128-wide one.
                    # No dummies: each column tile clears has_written on
                    # its own 32-partition stripe with start=True on its ch0
                    # matmul (testing per-partition clear semantics).
                    prev = None
                    for ch in range(CH):
                        for t in range(NT):
                            par, xb = t // 2, t % 2
                            lo = 64 * xb + par
                            mm = nc.tensor.matmul(
                                ps[XB * t:XB * (t + 1), 0:bsz * JW],
                                in1s[:, ch, y, lo:lo + 2 * XB - 1:2],
                                in2p[:, ch, y2f:y2f + 2 * bsz - 1:2,
                                     lo:lo + 2 * JW - 1:2],
                                start=(ch == 0),
                                stop=(ch == CH - 1),
                                skip_group_check=True,
                                tile_position=(0, XB * t))
                            if prev is not None:
                                tile_rust.add_dep_helper(
                                    mm.ins, prev.ins, sync=False,
                                    reason="psum bank order")
                            prev = mm
                    # Per-batch evacuation, column-split DVE/ACT ~45/55
                    # (proportional to 0.96 vs 1.2 GHz) so both engines
                    # finish together and per-bank drain latency halves.
                    w = bsz * JW
                    wd = (w * 45 // 100) & ~3
                    ob = goff + di * JW
                    copy_dve(stage[:, ob:ob + wd], ps[:, 0:wd])
                    copy_act(stage[:, ob + wd:ob + w], ps[:, wd:w])
                    di += bsz

                goff += n_dy * JW

                if y in gend:
                    # Alternate the two HWDGE rings so consecutive dumps
                    # (and especially the epilogue pair) drain in parallel.
                    deng = nc.sync if (y // YGRP) % 2 == 0 else nc.scalar
                    deng.dma_start(dump_d[:, gbase:gbase + goff],
                                   stage[:, 0:goff])

    nc.compile()
    return nc, offs, total


def _get_nc():
    if "nc" not in _NC_CACHE:
        _NC_CACHE["nc"] = _build()
    return _NC_CACHE["nc"]


def _assemble(dump, offs):
    """Shear one core's fp16 Gram dump into [441, H, W] fp32."""
    out = np.zeros((D * D, H, W), np.float32)
    ks = np.arange(D)
    for y in range(H):
        n, off = offs[y]
        blk = np.ascontiguousarray(
            dump[:, off:off + n * JW]).astype(np.float32) / np.float32(C)
        blk = blk.reshape(128, n, JW)
        dys = np.array(_valid_dys(y))
        d_idx = (dys[:, None] * D + ks[None, :]).ravel()
        for t in range(NT):
            par, xb = t // 2, t % 2
            g = blk[XB * t:XB * (t + 1)]          # [32, n, 52]
            s = g.strides
            diag = np.lib.stride_tricks.as_strided(
                g, shape=(n, D, XB), strides=(s[1], s[2], s[0] + s[2]))
            xsl = slice(64 * xb + par, 64 * xb + par + 2 * XB, 2)
            out[d_idx, y, xsl] = diag.reshape(n * D, XB)
    return out


def kernel(input1: np.ndarray, input2: np.ndarray) -> np.ndarray:
    from concourse.bass_utils import run_bass_kernel_spmd

    nc, offs, total = _get_nc()
    in_maps = [
        {"in1": np.ascontiguousarray(input1[b]).astype(np.float16),
         "in2": np.ascontiguousarray(input2[b]).astype(np.float16)}
        for b in range(N_CORES)
    ]
    res = run_bass_kernel_spmd(nc, in_maps, list(range(N_CORES)))
    out = np.empty((B, D * D, H, W), np.float32)
    for b in range(N_CORES):
        out[b] = _assemble(res.results[b]["dump"], offs)
    return out



# revision 10
# speedup vs baseline: 2.0712x; 2.0712x over previous
"""FlowNetC correlation (nn_Correlation_27797028340332) on 8 TRN2 NeuronCores.

out[b, dy*21+dx, y, x] = mean_c in1[b,c,y,x] * in2p[b,c,y+2*dy, x+2*dx]
with in2p = zero-pad(in2, 20) and (dy, dx) over a 21x21 stride-2 grid.

Strategy (per core; data-parallel over batch B=8):
  - Inputs are cast to fp16 on the host and DMA'd straight into resident
    SBUF tiles (no on-chip cast), halving input HBM traffic vs fp32.
  - The per-pixel C=256 dot products are banded Gram matmuls on the
    TensorEngine, 4x column-tiled: the 128 stationary columns are split
    into 4 tiles of 32 (parity p x x-block b), each with its OWN moving
    stream covering only the 52 in2p columns its 21-wide output band
    needs (vs 84 for a 64-wide tile).  The 4 tiles run concurrently in
    the PE array (tile_position=(0,32t)), so a (y,dy) pair costs ~52
    moving columns per contraction chunk instead of 84, and the dumped
    Gram rectangle shrinks from 84 to 52 columns per displacement.
  - PSUM: per (y, dy-batch<=7) one 2KB bank holds [128, 52*bsz] fp32.
    A 1-column dummy matmul with start=True clears the bank's
    has_written bits (bank-wide clear), then all 8 real matmuls (4 tiles
    x 2 contraction chunks) run with start=False: first write to each
    element overwrites, second accumulates.  This permits concurrent
    tiles to share a bank without clobbering each other's bits.
  - The needed output band G[x, x+2k] (k=0..20) is a per-partition
    diagonal no engine can extract at line rate, so the 52-column Grams
    are cast to fp16 (DVE/ACT alternating) and dumped to DRAM; the shear
    is a numpy strided view on the host inside kernel().
"""

import numpy as np

B, C, H, W = 8, 256, 96, 128
PAD = 20
D = 21            # displacements per axis
CH = 2            # contraction chunks of 128
WPAD = W + 2 * PAD    # 168
XB = 32           # stationary columns per PE tile
JW = 52           # moving columns per tile (32 outputs + 20 band overhang)
NT = 4            # PE column tiles (2 parities x 2 x-blocks)
BSZ = 7           # dy's per PSUM bank (7*52 = 364 <= 512 fp32 bank)
NBANK = 3         # PSUM banks per y (ceil(21/7))
ROWBLK = 8
N_CORES = 8


def _valid_dys(y):
    """dy' indices with in-range source row y2 = y + 2*dy' - 20."""
    return [d for d in range(D) if 0 <= y + 2 * d - PAD < H]


def _batches(n):
    """Chunks of BSZ with remainder tail: n=21 -> (7,7,7); n=11 -> (7,4)."""
    out = [BSZ] * (n // BSZ)
    if n % BSZ:
        out.append(n % BSZ)
    return out


def _dump_layout():
    """Per-y (n_dy, element offset) layout of the dump tensor's free dim."""
    offs, off = [], 0
    for y in range(H):
        n = len(_valid_dys(y))
        offs.append((n, off))
        off += n * JW
    return offs, off


_NC_CACHE = {}


def _build(reps=1):
    import contextlib

    import concourse.bacc as bacc
    import concourse.tile as tile
    import concourse.tile_rust as tile_rust
    from concourse import mybir

    offs, total = _dump_layout()

    nc = bacc.Bacc("TRN2", target_bir_lowering=False, debug=False)
    in1_d = nc.dram_tensor("in1", [C, H, W], mybir.dt.float16,
                           kind="ExternalInput").ap()
    in2_d = nc.dram_tensor("in2", [C, H, W], mybir.dt.float16,
                           kind="ExternalInput").ap()
    dump_d = nc.dram_tensor("dump", [128, total], mybir.dt.float16,
                            kind="ExternalOutput").ap()

    YGRP = 4                      # y rows per dump DMA
    MAXW = NBANK * BSZ * JW       # 1092 elems: worst-case per-y dump width

    with tile.TileContext(nc) as tc:
        with tc.tile_pool(name="resident", bufs=1) as res_pool, \
             tc.tile_pool(name="out", bufs=4) as out_pool, \
             tc.tile_pool(name="psum", bufs=8, space="PSUM") as psum_pool, \
             (tc.For_i(0, reps, 1) if reps > 1 else contextlib.nullcontext()):

            # Fully-resident fp16 feature maps; in2 zero-padded along x.
            in1s = res_pool.tile([128, CH, H, W], mybir.dt.float16)
            in2p = res_pool.tile([128, CH, H, WPAD], mybir.dt.float16)
            # Pad memsets split so y=0 only waits on the small rows-0-15
            # piece (~1.4us on DVE), not a whole-tensor gpsimd memset.
            PRE = 2 * ROWBLK
            nc.vector.memset(in2p[:, :, 0:PRE, 0:PAD], 0.0)
            nc.vector.memset(in2p[:, :, 0:PRE, W + PAD:WPAD], 0.0)
            nc.gpsimd.memset(in2p[:, :, PRE:H, 0:PAD], 0.0)
            nc.vector.memset(in2p[:, :, PRE:H, W + PAD:WPAD], 0.0)

            def load1_block(yb, eng=None):
                y0 = yb * ROWBLK
                (eng or nc.gpsimd).dma_start(
                    in1s[:, :, y0:y0 + ROWBLK, :],
                    in1_d[:, y0:y0 + ROWBLK, :].rearrange(
                        "(k p) y x -> p k y x", p=128))

            def load2_block(yb, eng=None):
                y0 = yb * ROWBLK
                for k in range(CH):
                    (eng or nc.gpsimd).dma_start(
                        in2p[:, k, y0:y0 + ROWBLK, PAD:PAD + W],
                        in2_d[128 * k:128 * (k + 1), y0:y0 + ROWBLK, :]
                        .rearrange("p y x -> p y x"))

            def load_block(yb):
                """DMA rows [yb*8, yb*8+8) of both inputs into SBUF."""
                load1_block(yb)
                load2_block(yb)

            # Prologue on the idle HWDGE rings (SP + ACT), y=0's deps
            # first (in1 rows 0-7, in2 rows 0-23), then the rest of the
            # 4-block lookahead window; steady-state loads use SWDGE.
            # (Splitting the first in1 load to row-0-only starts y=0
            # ~1.1us earlier but starves y=1-4 — net worse; measured.)
            load1_block(0, nc.sync)
            load2_block(0, nc.scalar)
            load2_block(1, nc.sync)
            load2_block(2, nc.scalar)
            load2_block(3, nc.sync)
            for yb in range(1, 4):
                load1_block(yb)

            def copy_dve(out, in_):
                nc.vector.tensor_copy(out, in_)

            def copy_act(out, in_):
                nc.scalar.copy(out, in_)

            # Dump groups of 4 y's, except the last 8 y's in pairs so the
            # final copies+DMA tail after the last matmul stays short.
            gstart = set(range(0, H - 8, YGRP)) | set(range(H - 8, H, 2))
            gend = {y - 1 for y in gstart if y > 0} | {H - 1}

            stage = None
            goff = 0
            gbase = 0
            for y in range(H):
                # Stay 3-4 blocks ahead of the in2 read frontier (y+20).
                if y % ROWBLK == 0:
                    yb = y // ROWBLK + 4
                    if yb < H // ROWBLK:
                        load_block(yb)

                if y in gstart:
                    stage = out_pool.tile([128, YGRP * MAXW],
                                          mybir.dt.float16, tag="dumpstage")
                    goff = 0
                    gbase = offs[y][1]

                dys = _valid_dys(y)
                n_dy = len(dys)
                bs = _batches(n_dy)

                di = 0
                for ib, bsz in enumerate(bs):
                    # One PSUM bank per dy-batch, 6 in flight: matmuls for
                    # later batches never wait on this batch's evacuation,
                    # so the PE stays busy (and in its fast p-state) while
                    # DVE/ACT drain earlier banks.
                    ps = psum_pool.tile([128, 512], mybir.dt.float32,
                                        tag="ps")
                    dy0 = dys[di]
                    y2f = y + 2 * dy0 - PAD
                    # Dummy 1-col matmuls, one per column tile: start=True
                    # clears the bank's has_written bits (their union spans
                    # all 128 partitions); they write only col 511 (never
                    # read).  Real matmuls then use start=False so the 4
                    # concurrent column tiles can share the bank.  Col-tiled
                    # dummies chain behind their own group's stream instead
                    # of barriering the whole array like a 128-wide one.
                    # No dummies: each column tile clears has_written on
                    # its own 32-partition stripe with start=True on its ch0
                    # matmul (testing per-partition clear semantics).
                    prev = None
                    for ch in range(CH):
                        for t in range(NT):
                            par, xb = t // 2, t % 2
                            lo = 64 * xb + par
                            mm = nc.tensor.matmul(
                                ps[XB * t:XB * (t + 1), 0:bsz * JW],
                                in1s[:, ch, y, lo:lo + 2 * XB - 1:2],
                                in2p[:, ch, y2f:y2f + 2 * bsz - 1:2,
                                     lo:lo + 2 * JW - 1:2],
                                start=(ch == 0),
                                stop=(ch == CH - 1),
                                skip_group_check=True,
                                tile_position=(0, XB * t))
                            if prev is not None:
                                tile_rust.add_dep_helper(
                                    mm.ins, prev.ins, sync=False,
                                    reason="psum bank order")
                            prev = mm
                    # Per-batch evacuation, column-split DVE/ACT ~45/55
                    # (proportional to 0.96 vs 1.2 GHz) so both engines
                    # finish together and per-bank drain latency halves.
                    w = bsz * JW
                    wd = (w * 45 // 100) & ~3
                    ob = goff + di * JW
                    copy_dve(stage[:, ob:ob + wd], ps[:, 0:wd])
                    copy_act(stage[:, ob + wd:ob + w], ps[:, wd:w])
                    di += bsz

                goff += n_dy * JW

                if y in gend:
                    # Alternate the two HWDGE rings so consecutive dumps
                    # (and especially the epilogue pair) drain in parallel.
                    deng = nc.sync if (y // YGRP) % 2 == 0 else nc.scalar
                    deng.dma_start(dump_d[:, gbase:gbase + goff],
                                   stage[:, 0:goff])

    nc.compile()
    return nc, offs, total


def _get_nc():
    if "nc" not in _NC_CACHE:
        _NC_CACHE["nc"] = _build()
    return _NC_CACHE["nc"]


def _assemble(dump, offs):
    """Shear one core's fp16 Gram dump into [441, H, W] fp32."""
    out = np.zeros((D * D, H, W), np.float32)
    ks = np.arange(D)
    for y in range(H):
        n, off = offs[y]
        blk = np.ascontiguousarray(
            dump[:, off:off + n * JW]).astype(np.float32) / np.float32(C)
        blk = blk.reshape(128, n, JW)
        dys = np.array(_valid_dys(y))
        d_idx = (dys[:, None] * D + ks[None, :]).ravel()
        for t in range(NT):
            par, xb = t // 2, t % 2
            g = blk[XB * t:XB * (t + 1)]          # [32, n, 52]
            s = g.strides
            diag = np.lib.stride_tricks.as_strided(
                g, shape=(n, D, XB), strides=(s[1], s[2], s[0] + s[2]))
            xsl = slice(64 * xb + par, 64 * xb + par + 2 * XB, 2)
            out[d_idx, y, xsl] = diag.reshape(n * D, XB)
    return out


def kernel(input1: np.ndarray, input2: np.ndarray) -> np.ndarray:
    from concourse.bass_utils import run_bass_kernel_spmd

    nc, offs, total = _get_nc()
    in_maps = [
        {"in1": np.ascontiguousarray(input1[b]).astype(np.float16),
         "in2": np.ascontiguousarray(input2[b]).astype(np.float16)}
        for b in range(N_CORES)
    ]
    res = run_bass_kernel_spmd(nc, in_maps, list(range(N_CORES)))
    out = np.empty((B, D * D, H, W), np.float32)
    for b in range(N_CORES):
        out[b] = _assemble(res.results[b]["dump"], offs)
    return out

